# revision 1
# baseline (speedup 1.0000x reference)
"""Multi-head GAT layer on 8 Trainium2 NeuronCores.

Reference (B=4, N=2048, IN=256, H=4, D=64):
    q = (h @ W).reshape(B,N,H,D)
    e[b,i,j,h] = leakyrelu(q[b,i,h]@a_src + q[b,j,h]@a_dst, 0.2)
    attn = softmax_j(where(adj[i,j], e, -9e15))
    out  = elu(einsum('bijh,bjhd->bihd', attn, q).reshape(B,N,H*D))

Sharding: 16 (b,h) pairs -> 2 pairs per core (same b, adjacent heads).
Each core holds all N query rows for its two heads.

Key math used on device (per (b,h)):
  s_i = q_i . a_src, d_j = q_j . a_dst  (rank-1 projections, host-computed)
  With P[j,i] layout (keys on partitions):
    x  = s_i + d_j + 150*(adjT[j,i]-1)     (one fused DVE op; mask folded in)
    y  = max(0.2x, x)                      (LeakyReLU; masked entries -> ~-30)
    P  = exp(y)                            (ACT; masked weights ~1e-13 -> 0)
  numerator+denominator in one PSUM chain: [P^T @ [q_h | 1]] accumulated
  over key tiles; epilogue divides and applies ELU.

Softmax max-subtraction is skipped: e ~ N(0,1) here so exp() cannot
overflow, and softmax is shift-invariant so the result matches the
reference to float rounding.
"""

import numpy as np
import ml_dtypes

B, N, IN_DIM, H, D = 4, 2048, 256, 4, 64
ALPHA = 0.2
MASK_SCALE = 150.0  # lrelu(-150+e) ~ -30 -> exp ~ 1e-13 ~ 0
NCORES = 8
P = 128
NJT = N // P  # 16 key tiles
BF16 = ml_dtypes.bfloat16

_CACHE = {}
RUN_OPTS = {"trace": False}


def _build_bass():
    import concourse.bass as bass
    import concourse.mybir as mybir
    from concourse import bacc
    from concourse.tile import TileContext

    f32 = mybir.dt.float32
    f32r = mybir.dt.float32r
    bf16 = mybir.dt.bfloat16
    Alu = mybir.AluOpType
    Act = mybir.ActivationFunctionType

    nc = bacc.Bacc("TRN2", target_bir_lowering=False, debug=False, num_devices=NCORES)

    hT = nc.dram_tensor("hT", [IN_DIM, N], f32r, kind="ExternalInput")
    Wc = nc.dram_tensor("Wc", [IN_DIM, 256], f32r, kind="ExternalInput")
    adjm = nc.dram_tensor("adjm", [N, N], bf16, kind="ExternalInput")
    sT = nc.dram_tensor("sT", [2, N], bf16, kind="ExternalInput")
    dk = nc.dram_tensor("dk", [P, NJT, 2], f32, kind="ExternalInput")
    o = nc.dram_tensor("o", [N, 2 * D], f32, kind="ExternalOutput")

    with TileContext(nc) as tc:
        with (
            tc.tile_pool(name="singles", bufs=1) as singles,
            tc.tile_pool(name="xp", bufs=3) as xp,
            tc.tile_pool(name="yp", bufs=2) as yp,
            tc.tile_pool(name="pp", bufs=5) as pp,
            tc.tile_pool(name="psq", bufs=2, space="PSUM") as psq,
            tc.tile_pool(name="acc", bufs=1, space="PSUM") as accp,
            tc.tile_pool(name="tpp", bufs=2, space="PSUM") as tpp,
            tc.tile_pool(name="epi", bufs=1) as epi,
        ):
            # ---- resident loads (issue order = DMA priority) ----
            d_sb = singles.tile([P, NJT, 2], f32, tag="d")
            nc.sync.dma_start(out=d_sb, in_=dk[:])
            s_bc = []
            for hl in range(2):
                t = singles.tile([P, N], bf16, tag=f"s{hl}", name=f"s{hl}")
                row = sT[hl : hl + 1, :]
                brow = bass.AP(tensor=row.tensor, offset=row.offset,
                               ap=[[0, P]] + list(row.ap[1:]))
                nc.sync.dma_start(out=t, in_=brow)
                s_bc.append(t)
            adj_sb = [singles.tile([P, N], bf16, tag=f"adj{jt}", name=f"adj{jt}")
                      for jt in range(NJT)]
            for jt in range(4):
                for hf in range(2):
                    nc.sync.dma_start(
                        out=adj_sb[jt][hf * 64 : (hf + 1) * 64, :],
                        in_=adjm[jt * P + hf * 64 : jt * P + (hf + 1) * 64, :])
            w_sb = singles.tile([P, 2, 256], f32r, tag="w")
            nc.sync.dma_start(out=w_sb, in_=Wc[:].rearrange("(a p) c -> p a c", p=P))
            h_sb = singles.tile([P, 2, N], f32r, tag="h")
            hTv = hT[:].rearrange("(a p) j -> p a j", p=P)
            for jt in range(NJT):
                nc.sync.dma_start(out=h_sb[:, :, jt * P : (jt + 1) * P],
                                  in_=hTv[:, :, jt * P : (jt + 1) * P])
            for jt in range(4, NJT):
                nc.sync.dma_start(out=adj_sb[jt], in_=adjm[jt * P : (jt + 1) * P, :])

            # ---- q generation: q[j, c] for this core's batch, 2 heads ----
            # Wc columns: [W_h0 (64) | W_h1 (64) | zeros(128)] so psum cols
            # 0:128 are the two heads' q values.
            vp_sb = []
            for jt in range(NJT):
                qp = psq.tile([P, 256], f32)
                for half in range(2):
                    nc.tensor.matmul(
                        qp,
                        lhsT=h_sb[:, half, jt * P : (jt + 1) * P],
                        rhs=w_sb[:, half, :],
                        start=(half == 0),
                        stop=(half == 1),
                    )
                vp = singles.tile([P, 2, 65], bf16, tag=f"vp{jt}")
                nc.scalar.copy(
                    out=vp[:, :, 0:64],
                    in_=qp[:, 0:128].rearrange("p (a c) -> p a c", a=2),
                )
                nc.gpsimd.memset(vp[:, :, 64:65], 1.0)
                vp_sb.append(vp)

            # ---- attention per local head ----
            ident = singles.tile([65, 65], bf16, tag="ident")
            from concourse.masks import make_identity
            make_identity(nc, ident)
            for hl in range(2):
                # acc[c, i]: rows 0:64 = numerator^T, row 64 = denominator^T.
                # Each 512-wide f32 slice fills exactly one PSUM bank = one
                # accumulation group (groups are per-bank on TRN2).
                acc = accp.tile([65, N], f32, name="acc")
                for jt in range(NJT):
                    d_col = d_sb[:, jt, hl : hl + 1]
                    if jt % 3 == 1 and jt < 15:
                        y = yp.tile([P, N], bf16, name="y")
                        # ACT route: bias-fused broadcast-add + LeakyReLU
                        # (Prelu honors alpha on HW; Lrelu hardcodes 0.01)
                        nc.scalar.activation(out=y, in_=s_bc[hl], func=Act.Prelu,
                                             bias=d_col, alpha=ALPHA)
                    else:
                        # DVE route (4x/4x/2x modes)
                        t1 = xp.tile([P, N], bf16, tag="t1")
                        nc.vector.tensor_scalar(t1, s_bc[hl], d_col, None, Alu.add)
                        y = xp.tile([P, N], bf16, tag="xs", name="y")
                        nc.vector.tensor_scalar(y, t1, ALPHA, None, Alu.mult)
                        nc.vector.tensor_tensor(out=y, in0=y, in1=t1, op=Alu.max)
                    u = pp.tile([P, N], bf16, tag="u")
                    nc.scalar.activation(out=u, in_=y, func=Act.Exp)
                    pt = pp.tile([P, N], bf16, tag="pt")
                    nc.vector.tensor_tensor(out=pt, in0=u, in1=adj_sb[jt], op=Alu.mult)
                    for sl in range(4):
                        nc.tensor.matmul(
                            acc[:, sl * 512 : (sl + 1) * 512],
                            lhsT=vp_sb[jt][:, hl, :],
                            rhs=pt[:, sl * 512 : (sl + 1) * 512],
                            start=(jt == 0),
                            stop=(jt == NJT - 1),
                        )
                # epilogue: transpose [65, N] -> [N, 65], divide, ELU, store
                cp = epi.tile([65, N], bf16, tag="cp")
                nc.scalar.copy(out=cp[:, 0 : N // 2], in_=acc[:, 0 : N // 2])
                nc.vector.tensor_copy(out=cp[:, N // 2 : N], in_=acc[:, N // 2 : N])
                zt = epi.tile([P, NJT, 65], bf16, tag="zt")
                for t in range(NJT):
                    tp = tpp.tile([P, 65], bf16)
                    nc.tensor.transpose(tp, cp[:, t * P : (t + 1) * P], ident)
                    if t % 2 == 0:
                        nc.scalar.copy(out=zt[:, t, :], in_=tp)
                    else:
                        nc.vector.tensor_copy(out=zt[:, t, :], in_=tp)
                rec = epi.tile([P, NJT], f32, tag="rec")
                nc.vector.reciprocal(out=rec, in_=zt[:, :, 64])
                zz = epi.tile([P, NJT, 64], f32, tag="zz")
                rb = bass.AP(tensor=rec.tensor, offset=rec.offset,
                             ap=list(rec.ap) + [[0, 64]])
                nc.vector.tensor_tensor(out=zz, in0=zt[:, :, 0:64], in1=rb,
                                        op=Alu.mult)
                zm = epi.tile([P, NJT, 64], f32, tag="zm")
                nc.vector.tensor_scalar_min(zm, zz, 0.0)
                em = epi.tile([P, NJT, 64], f32, tag="em")
                nc.scalar.activation(out=em, in_=zm, func=Act.Exp)
                fin = epi.tile([P, NJT, 64], f32, tag="fin")
                nc.vector.scalar_tensor_tensor(
                    out=fin, in0=em, scalar=-1.0, in1=zz,
                    op0=Alu.add, op1=Alu.max,
                )
                ov = o[:].rearrange("(t p) c -> p t c", p=P)
                nc.sync.dma_start(
                    out=ov[:, :, hl * 64 : (hl + 1) * 64], in_=fin,
                )
    nc.finalize()
    return nc


def kernel(h, adj, W, a):
    from concourse import bass_utils

    h = np.asarray(h, dtype=np.float32)
    adj = np.asarray(adj)
    W = np.asarray(W, dtype=np.float32)
    a = np.asarray(a, dtype=np.float32)

    # host prep: rank-1 projections + transposed/masked views
    Wr = W.reshape(IN_DIM, H, D)
    ws = np.einsum("khd,d->kh", Wr, a[:D]).astype(np.float32)
    wd = np.einsum("khd,d->kh", Wr, a[D:]).astype(np.float32)
    s_all = (h @ ws).astype(np.float32)  # [B,N,H]
    d_all = (h @ wd).astype(np.float32)  # [B,N,H]
    adjm = adj.T.astype(BF16)
    hT = np.ascontiguousarray(h.transpose(0, 2, 1))  # [B,IN,N]

    if "nc" not in _CACHE:
        _CACHE["nc"] = _build_bass()
    nc = _CACHE["nc"]

    in_maps = []
    for c in range(NCORES):
        b, pair = divmod(c, 2)
        h0 = 2 * pair
        Wc = np.zeros((IN_DIM, 256), dtype=np.float32)
        Wc[:, :128] = W[:, h0 * D : (h0 + 2) * D]
        sT = np.ascontiguousarray(s_all[b][:, h0 : h0 + 2].T).astype(BF16)
        dkv = np.ascontiguousarray(
            d_all[b][:, h0 : h0 + 2].reshape(NJT, P, 2).transpose(1, 0, 2)
        ).astype(np.float32)
        in_maps.append(
            {"hT": np.ascontiguousarray(hT[b]), "Wc": Wc, "adjm": adjm,
             "sT": sT, "dk": dkv}
        )

    res = bass_utils.run_bass_kernel_spmd(
        nc, in_maps, core_ids=list(range(NCORES)), trace=RUN_OPTS.get("trace", False),
    )
    _CACHE["last_results"] = res

    out = np.empty((B, N, H * D), dtype=np.float32)
    for c in range(NCORES):
        b, pair = divmod(c, 2)
        out[b, :, pair * 128 : (pair + 1) * 128] = res.results[c]["o"]
    return out



# revision 6
# speedup vs baseline: 1.1676x; 1.1676x over previous
"""Multi-head GAT layer on 8 Trainium2 NeuronCores.

Reference (B=4, N=2048, IN=256, H=4, D=64):
    q = (h @ W).reshape(B,N,H,D)
    e[b,i,j,h] = leakyrelu(q[b,i,h]@a_src + q[b,j,h]@a_dst, 0.2)
    attn = softmax_j(where(adj[i,j], e, -9e15))
    out  = elu(einsum('bijh,bjhd->bihd', attn, q).reshape(B,N,H*D))

Sharding: 16 (b,h) pairs -> 2 pairs per core (same b, adjacent heads).
Each core holds all N query rows for its two heads.

Key math (per (b,h), exp monotone + softmax row-scale invariance):
  exp(lrelu(x)) = max(e^x, e^{0.2x});  x = s_i + d_j
  P_ij = adj_ij * max(e^x, e^{0.2x})
       = v_i * [ adj_ij * B_j * max(r_i*rho_j, 1) ]      (v_i cancels)
  with r=e^{0.8s}, rho=e^{0.8d}, B=e^{0.2d} precomputed on host (O(N)).
  Device per 128-key tile (layout P[j,i], keys on partitions):
    DVE route: T  = (r_bc * rho_j) max 1          (one 4x tensor_scalar)
               pt = (T * B_j) * adj               (one fused STT)
    ACT route: t  = Relu(0.8*s_bc + 0.8*d_j)      (ACT, bias-fused)
               u  = Exp(t + 0.2*d_j)  = B_j*T     (ACT, bias-fused)
               pt = u * adj                       (one TT)
  numerator+denominator in one PSUM chain: [pt^T @ [q | 1]] accumulated
  over key tiles -> acc[65, N] f32, DMA'd out raw; the division by the
  denominator row and the ELU happen on host during unshard (softmax
  scale-invariance makes the v_i row scale drop out).
"""

import numpy as np
import ml_dtypes

B, N, IN_DIM, H, D = 4, 2048, 256, 4, 64
NCORES = 8
P = 128
NJT = N // P  # 16 key tiles
BF16 = ml_dtypes.bfloat16
# key tiles routed to the ACT engine (per head) to balance DVE/ACT
ACT_TILES = frozenset((1, 3, 5, 7, 9, 11, 13))

_CACHE = {}
RUN_OPTS = {"trace": False}


def _build_bass():
    import concourse.bass as bass
    import concourse.mybir as mybir
    from concourse import bacc
    from concourse.tile import TileContext

    f32 = mybir.dt.float32
    bf16 = mybir.dt.bfloat16
    Alu = mybir.AluOpType
    Act = mybir.ActivationFunctionType

    nc = bacc.Bacc("TRN2", target_bir_lowering=False, debug=False, num_devices=NCORES)

    hT = nc.dram_tensor("hT", [IN_DIM, N], bf16, kind="ExternalInput")
    Wc = nc.dram_tensor("Wc", [IN_DIM, P], bf16, kind="ExternalInput")
    adjm = nc.dram_tensor("adjm", [N, N], bf16, kind="ExternalInput")
    sT = nc.dram_tensor("sT", [2, N], bf16, kind="ExternalInput")
    rT = nc.dram_tensor("rT", [2, N], bf16, kind="ExternalInput")
    dk = nc.dram_tensor("dk", [P, NJT, 2, 4], f32, kind="ExternalInput")
    o = nc.dram_tensor("o", [2, 65, N], f32, kind="ExternalOutput")

    def bcast_row(row):
        return bass.AP(tensor=row.tensor, offset=row.offset,
                       ap=[[0, P]] + list(row.ap[1:]))

    with TileContext(nc) as tc:
        with (
            tc.tile_pool(name="singles", bufs=1) as singles,
            tc.tile_pool(name="xp", bufs=4) as xp,
            tc.tile_pool(name="pp", bufs=4) as pp,
            tc.tile_pool(name="psq", bufs=2, space="PSUM") as psq,
            tc.tile_pool(name="accp", bufs=1, space="PSUM") as accp,
        ):
            # ---- resident loads (issue order = DMA priority) ----
            d_sb = singles.tile([P, NJT, 2, 4], f32, tag="d")
            nc.sync.dma_start(out=d_sb, in_=dk[:])
            s_bc, r_bc = [], []
            for hl in range(2):
                t = singles.tile([P, N], bf16, tag=f"s{hl}", name=f"s{hl}")
                nc.sync.dma_start(out=t, in_=bcast_row(sT[hl : hl + 1, :]))
                s_bc.append(t)
                t = singles.tile([P, N], bf16, tag=f"r{hl}", name=f"r{hl}")
                nc.sync.dma_start(out=t, in_=bcast_row(rT[hl : hl + 1, :]))
                r_bc.append(t)
            adj_sb = [singles.tile([P, N], bf16, tag=f"adj{jt}", name=f"adj{jt}")
                      for jt in range(NJT)]
            for jt in range(4):
                for hf in range(2):
                    nc.sync.dma_start(
                        out=adj_sb[jt][hf * 64 : (hf + 1) * 64, :],
                        in_=adjm[jt * P + hf * 64 : jt * P + (hf + 1) * 64, :])
            w_sb = singles.tile([P, 2, P], bf16, tag="w")
            nc.sync.dma_start(out=w_sb, in_=Wc[:].rearrange("(a p) c -> p a c", p=P))
            h_sb = singles.tile([P, 2, N], bf16, tag="h")
            hTv = hT[:].rearrange("(a p) j -> p a j", p=P)
            for jt in range(NJT):
                nc.sync.dma_start(out=h_sb[:, :, jt * P : (jt + 1) * P],
                                  in_=hTv[:, :, jt * P : (jt + 1) * P])
            for jt in range(4, NJT):
                nc.sync.dma_start(out=adj_sb[jt], in_=adjm[jt * P : (jt + 1) * P, :])

            # ---- q generation: vp[j, jt, hl, c] = [q | 1] per head ----
            vp = singles.tile([P, NJT, 2, 65], bf16, tag="vp")
            nc.gpsimd.memset(vp[:, :, :, 64:65], 1.0)
            for jt in range(NJT):
                qp = psq.tile([P, P], f32)
                for half in range(2):
                    nc.tensor.matmul(
                        qp,
                        lhsT=h_sb[:, half, jt * P : (jt + 1) * P],
                        rhs=w_sb[:, half, :],
                        start=(half == 0),
                        stop=(half == 1),
                    )
                nc.scalar.copy(
                    out=vp[:, jt, :, 0:64],
                    in_=qp.rearrange("p (a c) -> p a c", a=2),
                )

            # ---- attention per local head ----
            for hl in range(2):
                # acc[c, i]: rows 0:64 = numerator^T, row 64 = denominator^T.
                # Each 512-wide f32 slice fills exactly one PSUM bank = one
                # accumulation group (groups are per-bank on TRN2).
                acc = accp.tile([65, N], f32, name="acc")
                for jt in range(NJT):
                    rho = d_sb[:, jt, hl, 0:1]
                    Bv = d_sb[:, jt, hl, 1:2]
                    b08 = d_sb[:, jt, hl, 2:3]
                    b02 = d_sb[:, jt, hl, 3:4]
                    pt = pp.tile([P, N], bf16, tag="pt")
                    if jt in ACT_TILES:
                        t = xp.tile([P, N], bf16, tag="t")
                        nc.scalar.activation(out=t, in_=s_bc[hl], func=Act.Relu,
                                             bias=b08, scale=0.8)
                        u = xp.tile([P, N], bf16, tag="u")
                        nc.scalar.activation(out=u, in_=t, func=Act.Exp, bias=b02)
                        nc.vector.tensor_tensor(out=pt, in0=u, in1=adj_sb[jt],
                                                op=Alu.mult)
                    else:
                        T = xp.tile([P, N], bf16, tag="T")
                        nc.vector.tensor_scalar(T, r_bc[hl], rho, 1.0,
                                                Alu.mult, Alu.max)
                        nc.vector.scalar_tensor_tensor(
                            out=pt, in0=T, scalar=Bv, in1=adj_sb[jt],
                            op0=Alu.mult, op1=Alu.mult)
                    for sl in range(4):
                        nc.tensor.matmul(
                            acc[:, sl * 512 : (sl + 1) * 512],
                            lhsT=vp[:, jt, hl, :],
                            rhs=pt[:, sl * 512 : (sl + 1) * 512],
                            start=(jt == 0),
                            stop=(jt == NJT - 1),
                        )
                cp = pp.tile([65, N], f32, tag="cp", name="cp")
                nc.scalar.copy(out=cp[:, 0 : N // 2], in_=acc[:, 0 : N // 2])
                nc.vector.tensor_copy(out=cp[:, N // 2 : N], in_=acc[:, N // 2 : N])
                nc.sync.dma_start(out=o[hl], in_=cp)
    nc.finalize()
    return nc


def kernel(h, adj, W, a):
    from concourse import bass_utils

    h = np.asarray(h, dtype=np.float32)
    adj = np.asarray(adj)
    W = np.asarray(W, dtype=np.float32)
    a = np.asarray(a, dtype=np.float32)

    # host prep: rank-1 projections -> per-node exponentials (O(N) per head)
    Wr = W.reshape(IN_DIM, H, D)
    ws = np.einsum("khd,d->kh", Wr, a[:D]).astype(np.float32)
    wd = np.einsum("khd,d->kh", Wr, a[D:]).astype(np.float32)
    s_all = (h @ ws).astype(np.float32)  # [B,N,H]
    d_all = (h @ wd).astype(np.float32)  # [B,N,H]
    adjm = adj.T.astype(BF16)
    hTb = np.ascontiguousarray(h.transpose(0, 2, 1)).astype(BF16)  # [B,IN,N]

    if "nc" not in _CACHE:
        _CACHE["nc"] = _build_bass()
    nc = _CACHE["nc"]

    in_maps = []
    for c in range(NCORES):
        b, pair = divmod(c, 2)
        h0 = 2 * pair
        s2 = s_all[b][:, h0 : h0 + 2]  # [N, 2]
        d2 = d_all[b][:, h0 : h0 + 2]  # [N, 2]
        sTv = np.ascontiguousarray(s2.T).astype(BF16)
        rTv = np.ascontiguousarray(np.exp(0.8 * s2.T)).astype(BF16)
        # dk[p, jt, hl, (rho, B, 0.8d, 0.2d)]
        dkv = np.stack(
            [np.exp(0.8 * d2), np.exp(0.2 * d2), 0.8 * d2, 0.2 * d2], axis=-1
        ).reshape(NJT, P, 2, 4).transpose(1, 0, 2, 3)
        in_maps.append(
            {"hT": np.ascontiguousarray(hTb[b]),
             "Wc": np.ascontiguousarray(W[:, h0 * D : (h0 + 2) * D]).astype(BF16),
             "adjm": adjm, "sT": sTv, "rT": rTv,
             "dk": np.ascontiguousarray(dkv).astype(np.float32)}
        )

    res = bass_utils.run_bass_kernel_spmd(
        nc, in_maps, core_ids=list(range(NCORES)), trace=RUN_OPTS.get("trace", False),
    )
    _CACHE["last_results"] = res

    # unshard + epilogue: divide by denominator row, ELU (row scale of the
    # softmax cancels here, so the on-device v_i^-1 scaling is harmless)
    out = np.empty((B, N, H * D), dtype=np.float32)
    for c in range(NCORES):
        b, pair = divmod(c, 2)
        oc = res.results[c]["o"]  # [2, 65, N] f32
        for hl in range(2):
            num = oc[hl, 0:64, :]  # [64, N]
            den = oc[hl, 64, :]  # [N]
            z = (num / den).T  # [N, 64]
            col = (2 * pair + hl) * D
            out[b, :, col : col + D] = np.where(z > 0, z, np.expm1(z))
    return out


# revision 7
# speedup vs baseline: 1.2324x; 1.0555x over previous
"""Multi-head GAT layer on 8 Trainium2 NeuronCores.

Reference (B=4, N=2048, IN=256, H=4, D=64):
    q = (h @ W).reshape(B,N,H,D)
    e[b,i,j,h] = leakyrelu(q[b,i,h]@a_src + q[b,j,h]@a_dst, 0.2)
    attn = softmax_j(where(adj[i,j], e, -9e15))
    out  = elu(einsum('bijh,bjhd->bihd', attn, q).reshape(B,N,H*D))

Sharding: 16 (b,h) pairs -> 2 pairs per core (same b, adjacent heads).
Each core holds all N query rows for its two heads.

Key math (per (b,h)): with x = s_i + d_j (s_i = q_i.a_src, d_j = q_j.a_dst),
  exp(lrelu(x)) = max(e^x, e^0.2x)  (exp is monotone)
               = v_i * max(r_i*e^{d_j}, B_j),   r=e^{0.8s}, v=e^{0.2s}, B=e^{0.2d}
The row factor v_i cancels in the softmax, so the device computes the
v-scaled scores directly from host-precomputed O(N) exponential vectors:
  DVE route: T  = (r_bc * e^d_j) max B_j      (ONE 4x-mode tensor_scalar:
                                               per-partition scalar1+scalar2)
             pt = T * adj                      (one 2x-mode tensor_tensor)
  ACT route: t  = Relu(0.8*s_bc + 0.8*d_j)    (ACT, bias-fused)
             u  = Exp(t + 0.2*d_j)  = B_j*T   (ACT, bias-fused)
             pt = u * adj                     (one TT)
numerator+denominator in one PSUM chain: [pt^T @ [q | 1]] accumulated over
key tiles -> acc[65, N] f32 -> SBUF -> HBM raw; the divide by the
denominator row and the ELU run on host during unshard.
"""

import numpy as np
import ml_dtypes

B, N, IN_DIM, H, D = 4, 2048, 256, 4, 64
NCORES = 8
P = 128
NJT = N // P  # 16 key tiles
BF16 = ml_dtypes.bfloat16
# key tiles routed to the ACT engine (per head) to balance DVE vs ACT
ACT_TILES = frozenset((2, 5, 8, 11, 13, 15))
# key tiles whose mask-multiply runs on GpSimd (probe; per head)
GPS_TILES = frozenset((7,))

_CACHE = {}
RUN_OPTS = {"trace": False}


def _build_bass():
    import concourse.bass as bass
    import concourse.mybir as mybir
    from concourse import bacc
    from concourse.tile import TileContext

    f32 = mybir.dt.float32
    bf16 = mybir.dt.bfloat16
    Alu = mybir.AluOpType
    Act = mybir.ActivationFunctionType

    nc = bacc.Bacc("TRN2", target_bir_lowering=False, debug=False, num_devices=NCORES)

    hT = nc.dram_tensor("hT", [IN_DIM, N], bf16, kind="ExternalInput")
    Wc = nc.dram_tensor("Wc", [IN_DIM, P], bf16, kind="ExternalInput")
    adjm = nc.dram_tensor("adjm", [N, N], bf16, kind="ExternalInput")
    sT = nc.dram_tensor("sT", [2, N], bf16, kind="ExternalInput")
    rT = nc.dram_tensor("rT", [2, N], bf16, kind="ExternalInput")
    dk = nc.dram_tensor("dk", [P, NJT, 2, 4], f32, kind="ExternalInput")
    o = nc.dram_tensor("o", [2, 65, N], f32, kind="ExternalOutput")

    def bcast_row(row):
        return bass.AP(tensor=row.tensor, offset=row.offset,
                       ap=[[0, P]] + list(row.ap[1:]))

    with TileContext(nc) as tc:
        with (
            tc.tile_pool(name="singles", bufs=1) as singles,
            tc.tile_pool(name="xp", bufs=4) as xp,
            tc.tile_pool(name="pp", bufs=4) as pp,
            tc.tile_pool(name="psq", bufs=2, space="PSUM") as psq,
            tc.tile_pool(name="accp", bufs=1, space="PSUM") as accp,
        ):
            # ---- resident loads (issue order = DMA priority) ----
            # h/W first: qgen gates the whole attention pipeline.
            w_sb = singles.tile([P, 2, P], bf16, tag="w")
            nc.sync.dma_start(out=w_sb, in_=Wc[:].rearrange("(a p) c -> p a c", p=P))
            h_sb = singles.tile([P, 2, N], bf16, tag="h")
            hTv = hT[:].rearrange("(a p) j -> p a j", p=P)
            for jt in range(NJT):
                nc.sync.dma_start(out=h_sb[:, :, jt * P : (jt + 1) * P],
                                  in_=hTv[:, :, jt * P : (jt + 1) * P])
            d_sb = singles.tile([P, NJT, 2, 4], f32, tag="d")
            nc.sync.dma_start(out=d_sb, in_=dk[:])
            s_bc, r_bc = [], []
            for hl in range(2):
                t = singles.tile([P, N], bf16, tag=f"s{hl}", name=f"s{hl}")
                nc.sync.dma_start(out=t, in_=bcast_row(sT[hl : hl + 1, :]))
                s_bc.append(t)
                t = singles.tile([P, N], bf16, tag=f"r{hl}", name=f"r{hl}")
                nc.sync.dma_start(out=t, in_=bcast_row(rT[hl : hl + 1, :]))
                r_bc.append(t)
            adj_sb = [singles.tile([P, N], bf16, tag=f"adj{jt}", name=f"adj{jt}")
                      for jt in range(NJT)]
            for jt in range(2):
                for hf in range(2):
                    nc.sync.dma_start(
                        out=adj_sb[jt][hf * 64 : (hf + 1) * 64, :],
                        in_=adjm[jt * P + hf * 64 : jt * P + (hf + 1) * 64, :])
            for jt in range(2, NJT):
                nc.sync.dma_start(out=adj_sb[jt], in_=adjm[jt * P : (jt + 1) * P, :])

            # ---- q generation: vp[j, jt, hl, c] = [q | 1] per head ----
            vp = singles.tile([P, NJT, 2, 65], bf16, tag="vp")
            nc.gpsimd.memset(vp[:, :, :, 64:65], 1.0)
            for jt in range(NJT):
                qp = psq.tile([P, P], f32)
                for half in range(2):
                    nc.tensor.matmul(
                        qp,
                        lhsT=h_sb[:, half, jt * P : (jt + 1) * P],
                        rhs=w_sb[:, half, :],
                        start=(half == 0),
                        stop=(half == 1),
                    )
                nc.scalar.copy(
                    out=vp[:, jt, :, 0:64],
                    in_=qp.rearrange("p (a c) -> p a c", a=2),
                )

            # ---- attention per local head ----
            for hl in range(2):
                # acc[c, i]: rows 0:64 = numerator^T, row 64 = denominator^T.
                # Each 512-wide f32 slice fills exactly one PSUM bank = one
                # accumulation group (groups are per-bank on TRN2).
                acc = accp.tile([65, N], f32, name="acc")
                for jt in range(NJT):
                    ed = d_sb[:, jt, hl, 0:1]   # e^d
                    Bv = d_sb[:, jt, hl, 1:2]   # e^{0.2d}
                    b08 = d_sb[:, jt, hl, 2:3]  # 0.8d
                    b02 = d_sb[:, jt, hl, 3:4]  # 0.2d
                    pt = pp.tile([P, N], bf16, tag="pt")
                    if jt in ACT_TILES:
                        t = xp.tile([P, N], bf16, tag="t")
                        nc.scalar.activation(out=t, in_=s_bc[hl], func=Act.Relu,
                                             bias=b08, scale=0.8)
                        u = xp.tile([P, N], bf16, tag="u")
                        nc.scalar.activation(out=u, in_=t, func=Act.Exp, bias=b02)
                        nc.vector.tensor_tensor(out=pt, in0=u, in1=adj_sb[jt],
                                                op=Alu.mult)
                    else:
                        T = xp.tile([P, N], bf16, tag="T")
                        nc.vector.tensor_scalar(T, r_bc[hl], ed, Bv,
                                                Alu.mult, Alu.max)
                        eng = nc.gpsimd if jt in GPS_TILES else nc.vector
                        eng.tensor_tensor(out=pt, in0=T, in1=adj_sb[jt],
                                          op=Alu.mult)
                    for sl in range(4):
                        nc.tensor.matmul(
                            acc[:, sl * 512 : (sl + 1) * 512],
                            lhsT=vp[:, jt, hl, :],
                            rhs=pt[:, sl * 512 : (sl + 1) * 512],
                            start=(jt == 0),
                            stop=(jt == NJT - 1),
                        )
                cp = pp.tile([65, N], f32, tag="cp", name="cp")
                nc.scalar.copy(out=cp[:, 0 : N // 2], in_=acc[:, 0 : N // 2])
                nc.vector.tensor_copy(out=cp[:, N // 2 : N], in_=acc[:, N // 2 : N])
                nc.sync.dma_start(out=o[hl], in_=cp)
    nc.finalize()
    return nc


def kernel(h, adj, W, a):
    from concourse import bass_utils

    h = np.asarray(h, dtype=np.float32)
    adj = np.asarray(adj)
    W = np.asarray(W, dtype=np.float32)
    a = np.asarray(a, dtype=np.float32)

    # host prep: rank-1 projections -> per-node exponentials (O(N) per head)
    Wr = W.reshape(IN_DIM, H, D)
    ws = np.einsum("khd,d->kh", Wr, a[:D]).astype(np.float32)
    wd = np.einsum("khd,d->kh", Wr, a[D:]).astype(np.float32)
    s_all = (h @ ws).astype(np.float32)  # [B,N,H]
    d_all = (h @ wd).astype(np.float32)  # [B,N,H]
    adjm = adj.T.astype(BF16)
    hTb = np.ascontiguousarray(h.transpose(0, 2, 1)).astype(BF16)  # [B,IN,N]

    if "nc" not in _CACHE:
        _CACHE["nc"] = _build_bass()
    nc = _CACHE["nc"]

    in_maps = []
    for c in range(NCORES):
        b, pair = divmod(c, 2)
        h0 = 2 * pair
        s2 = s_all[b][:, h0 : h0 + 2]  # [N, 2]
        d2 = d_all[b][:, h0 : h0 + 2]  # [N, 2]
        sTv = np.ascontiguousarray(s2.T).astype(BF16)
        rTv = np.ascontiguousarray(np.exp(0.8 * s2.T)).astype(BF16)
        # dk[p, jt, hl, (e^d, e^{0.2d}, 0.8d, 0.2d)]
        dkv = np.stack(
            [np.exp(d2), np.exp(0.2 * d2), 0.8 * d2, 0.2 * d2], axis=-1
        ).reshape(NJT, P, 2, 4).transpose(1, 0, 2, 3)
        in_maps.append(
            {"hT": np.ascontiguousarray(hTb[b]),
             "Wc": np.ascontiguousarray(W[:, h0 * D : (h0 + 2) * D]).astype(BF16),
             "adjm": adjm, "sT": sTv, "rT": rTv,
             "dk": np.ascontiguousarray(dkv).astype(np.float32)}
        )

    res = bass_utils.run_bass_kernel_spmd(
        nc, in_maps, core_ids=list(range(NCORES)), trace=RUN_OPTS.get("trace", False),
    )
    _CACHE["last_results"] = res

    # unshard + epilogue: divide by denominator row, ELU (softmax row-scale
    # invariance makes the on-device v_i scaling drop out here)
    out = np.empty((B, N, H * D), dtype=np.float32)
    for c in range(NCORES):
        b, pair = divmod(c, 2)
        oc = res.results[c]["o"]  # [2, 65, N] f32
        for hl in range(2):
            num = oc[hl, 0:64, :]  # [64, N]
            den = oc[hl, 64, :]  # [N]
            z = (num / den).T  # [N, 64]
            col = (2 * pair + hl) * D
            out[b, :, col : col + D] = np.where(z > 0, z, np.expm1(z))
    return out


# revision 10
# speedup vs baseline: 1.3074x; 1.0609x over previous
"""Multi-head GAT layer on 8 Trainium2 NeuronCores.

Reference (B=4, N=2048, IN=256, H=4, D=64):
    q = (h @ W).reshape(B,N,H,D)
    e[b,i,j,h] = leakyrelu(q[b,i,h]@a_src + q[b,j,h]@a_dst, 0.2)
    attn = softmax_j(where(adj[i,j], e, -9e15))
    out  = elu(einsum('bijh,bjhd->bihd', attn, q).reshape(B,N,H*D))

Sharding: 16 (b,h) pairs -> 2 pairs per core (same b, adjacent heads).
Each core holds all N query rows for its two heads.

Key math (per (b,h)): with x = s_i + d_j (s_i = q_i.a_src, d_j = q_j.a_dst),
  exp(lrelu(x)) = max(e^x, e^0.2x)  (exp is monotone)
               = v_i * max(r_i*e^{d_j}, B_j),   r=e^{0.8s}, v=e^{0.2s}, B=e^{0.2d}
The row factor v_i cancels in the softmax, so the device computes the
v-scaled scores directly from host-precomputed O(N) exponential vectors:
  DVE route: T  = (r_bc * e^d_j) max B_j      (ONE 4x-mode tensor_scalar:
                                               per-partition scalar1+scalar2)
             pt = T * adj                      (one 2x-mode tensor_tensor)
  ACT route: t  = Relu(0.8*s_bc + 0.8*d_j)    (ACT, bias-fused)
             u  = Exp(t + 0.2*d_j)  = B_j*T   (ACT, bias-fused)
             pt = u * adj                     (one TT)
numerator+denominator in one PSUM chain: [pt^T @ [q | 1]] accumulated over
key tiles -> acc[65, N] f32 -> SBUF -> HBM raw; the divide by the
denominator row and the ELU run on host during unshard.
"""

import numpy as np
import ml_dtypes

B, N, IN_DIM, H, D = 4, 2048, 256, 4, 64
NCORES = 8
P = 128
NJT = N // P  # 16 key tiles
BF16 = ml_dtypes.bfloat16
# key tiles routed to the ACT engine (per head) to balance DVE vs ACT
ACT_TILES = frozenset((2, 4, 6, 9, 11, 13, 15))

_CACHE = {}
RUN_OPTS = {"trace": False}


def _build_bass():
    import concourse.bass as bass
    import concourse.mybir as mybir
    from concourse import bacc
    from concourse.tile import TileContext

    f32 = mybir.dt.float32
    bf16 = mybir.dt.bfloat16
    Alu = mybir.AluOpType
    Act = mybir.ActivationFunctionType

    nc = bacc.Bacc("TRN2", target_bir_lowering=False, debug=False, num_devices=NCORES)

    hT = nc.dram_tensor("hT", [IN_DIM, N], bf16, kind="ExternalInput")
    Wc = nc.dram_tensor("Wc", [IN_DIM, P], bf16, kind="ExternalInput")
    adjm = nc.dram_tensor("adjm", [N, N], bf16, kind="ExternalInput")
    sT = nc.dram_tensor("sT", [2, N], bf16, kind="ExternalInput")
    rT = nc.dram_tensor("rT", [2, N], bf16, kind="ExternalInput")
    dk = nc.dram_tensor("dk", [P, NJT, 2, 4], f32, kind="ExternalInput")
    o = nc.dram_tensor("o", [2, 65, N], f32, kind="ExternalOutput")

    def bcast_row(row):
        return bass.AP(tensor=row.tensor, offset=row.offset,
                       ap=[[0, P]] + list(row.ap[1:]))

    with TileContext(nc) as tc:
        with (
            tc.tile_pool(name="singles", bufs=1) as singles,
            tc.tile_pool(name="xp", bufs=4) as xp,
            tc.tile_pool(name="pp", bufs=4) as pp,
            tc.tile_pool(name="psq", bufs=2, space="PSUM") as psq,
            tc.tile_pool(name="accp", bufs=1, space="PSUM") as accp,
        ):
            # ---- resident loads (issue order = DMA priority) ----
            # h/W first: qgen gates the whole attention pipeline. Few, large
            # DMAs: each dma_start costs ~650ns of serialized issue time on
            # its queue, so 42 small DMAs would stall the pipeline ~28us.
            w_sb = singles.tile([P, 2, P], bf16, tag="w")
            nc.sync.dma_start(out=w_sb, in_=Wc[:].rearrange("(a p) c -> p a c", p=P))
            h_sb = singles.tile([P, 2, N], bf16, tag="h")
            nc.sync.dma_start(out=h_sb, in_=hT[:].rearrange("(a p) j -> p a j", p=P))
            d_sb = singles.tile([P, NJT, 2, 4], f32, tag="d")
            nc.sync.dma_start(out=d_sb, in_=dk[:])
            s_bc, r_bc = [], []
            for hl in range(2):
                t = singles.tile([P, N], bf16, tag=f"s{hl}", name=f"s{hl}")
                nc.sync.dma_start(out=t, in_=bcast_row(sT[hl : hl + 1, :]))
                s_bc.append(t)
                t = singles.tile([P, N], bf16, tag=f"r{hl}", name=f"r{hl}")
                nc.sync.dma_start(out=t, in_=bcast_row(rT[hl : hl + 1, :]))
                r_bc.append(t)
            # adjacency as one resident tile; grouped DMAs, first tiles solo
            # so head-0 compute starts early. Issue split across both HWDGE
            # queues (sync + scalar) to halve serial issue time.
            adj_big = singles.tile([P, NJT, N], bf16, tag="adj")
            adjv = adjm[:].rearrange("(t p) i -> p t i", p=P)
            adj_sb = [adj_big[:, jt, :] for jt in range(NJT)]
            nc.sync.dma_start(out=adj_big[:, 0:1, :], in_=adjv[:, 0:1, :])
            nc.scalar.dma_start(out=adj_big[:, 1:2, :], in_=adjv[:, 1:2, :])
            nc.sync.dma_start(out=adj_big[:, 2:4, :], in_=adjv[:, 2:4, :])
            nc.scalar.dma_start(out=adj_big[:, 4:6, :], in_=adjv[:, 4:6, :])
            nc.sync.dma_start(out=adj_big[:, 6:9, :], in_=adjv[:, 6:9, :])
            nc.scalar.dma_start(out=adj_big[:, 9:12, :], in_=adjv[:, 9:12, :])
            nc.sync.dma_start(out=adj_big[:, 12:16, :], in_=adjv[:, 12:16, :])

            # ---- q generation: vp[j, jt, hl, c] = [q | 1] per head ----
            vp = singles.tile([P, NJT, 2, 65], bf16, tag="vp")
            nc.gpsimd.memset(vp[:, :, :, 64:65], 1.0)
            for jt in range(NJT):
                qp = psq.tile([P, P], f32)
                for half in range(2):
                    nc.tensor.matmul(
                        qp,
                        lhsT=h_sb[:, half, jt * P : (jt + 1) * P],
                        rhs=w_sb[:, half, :],
                        start=(half == 0),
                        stop=(half == 1),
                    )
                nc.scalar.copy(
                    out=vp[:, jt, :, 0:64],
                    in_=qp.rearrange("p (a c) -> p a c", a=2),
                )

            # ---- attention per local head ----
            for hl in range(2):
                # acc[c, i]: rows 0:64 = numerator^T, row 64 = denominator^T.
                # Each 512-wide f32 slice fills exactly one PSUM bank = one
                # accumulation group (groups are per-bank on TRN2).
                acc = accp.tile([65, N], f32, name="acc")
                for jt in range(NJT):
                    ed = d_sb[:, jt, hl, 0:1]   # e^d
                    Bv = d_sb[:, jt, hl, 1:2]   # e^{0.2d}
                    b08 = d_sb[:, jt, hl, 2:3]  # 0.8d
                    b02 = d_sb[:, jt, hl, 3:4]  # 0.2d
                    pt = pp.tile([P, N], bf16, tag="pt")
                    if jt in ACT_TILES:
                        t = xp.tile([P, N], bf16, tag="t")
                        nc.scalar.activation(out=t, in_=s_bc[hl], func=Act.Relu,
                                             bias=b08, scale=0.8)
                        u = xp.tile([P, N], bf16, tag="u")
                        nc.scalar.activation(out=u, in_=t, func=Act.Exp, bias=b02)
                        nc.vector.tensor_tensor(out=pt, in0=u, in1=adj_sb[jt],
                                                op=Alu.mult)
                    else:
                        T = xp.tile([P, N], bf16, tag="T")
                        nc.vector.tensor_scalar(T, r_bc[hl], ed, Bv,
                                                Alu.mult, Alu.max)
                        nc.vector.tensor_tensor(out=pt, in0=T, in1=adj_sb[jt],
                                                op=Alu.mult)
                    for sl in range(4):
                        nc.tensor.matmul(
                            acc[:, sl * 512 : (sl + 1) * 512],
                            lhsT=vp[:, jt, hl, :],
                            rhs=pt[:, sl * 512 : (sl + 1) * 512],
                            start=(jt == 0),
                            stop=(jt == NJT - 1),
                        )
                cp = pp.tile([65, N], f32, tag="cp", name="cp")
                nc.scalar.copy(out=cp[:, 0 : N // 2], in_=acc[:, 0 : N // 2])
                nc.vector.tensor_copy(out=cp[:, N // 2 : N], in_=acc[:, N // 2 : N])
                nc.sync.dma_start(out=o[hl], in_=cp)
    nc.finalize()
    return nc


def kernel(h, adj, W, a):
    from concourse import bass_utils

    h = np.asarray(h, dtype=np.float32)
    adj = np.asarray(adj)
    W = np.asarray(W, dtype=np.float32)
    a = np.asarray(a, dtype=np.float32)

    # host prep: rank-1 projections -> per-node exponentials (O(N) per head)
    Wr = W.reshape(IN_DIM, H, D)
    ws = np.einsum("khd,d->kh", Wr, a[:D]).astype(np.float32)
    wd = np.einsum("khd,d->kh", Wr, a[D:]).astype(np.float32)
    s_all = (h @ ws).astype(np.float32)  # [B,N,H]
    d_all = (h @ wd).astype(np.float32)  # [B,N,H]
    adjm = adj.T.astype(BF16)
    hTb = np.ascontiguousarray(h.transpose(0, 2, 1)).astype(BF16)  # [B,IN,N]

    if "nc" not in _CACHE:
        _CACHE["nc"] = _build_bass()
    nc = _CACHE["nc"]

    in_maps = []
    for c in range(NCORES):
        b, pair = divmod(c, 2)
        h0 = 2 * pair
        s2 = s_all[b][:, h0 : h0 + 2]  # [N, 2]
        d2 = d_all[b][:, h0 : h0 + 2]  # [N, 2]
        sTv = np.ascontiguousarray(s2.T).astype(BF16)
        rTv = np.ascontiguousarray(np.exp(0.8 * s2.T)).astype(BF16)
        # dk[p, jt, hl, (e^d, e^{0.2d}, 0.8d, 0.2d)]
        dkv = np.stack(
            [np.exp(d2), np.exp(0.2 * d2), 0.8 * d2, 0.2 * d2], axis=-1
        ).reshape(NJT, P, 2, 4).transpose(1, 0, 2, 3)
        in_maps.append(
            {"hT": np.ascontiguousarray(hTb[b]),
             "Wc": np.ascontiguousarray(W[:, h0 * D : (h0 + 2) * D]).astype(BF16),
             "adjm": adjm, "sT": sTv, "rT": rTv,
             "dk": np.ascontiguousarray(dkv).astype(np.float32)}
        )

    res = bass_utils.run_bass_kernel_spmd(
        nc, in_maps, core_ids=list(range(NCORES)), trace=RUN_OPTS.get("trace", False),
    )
    _CACHE["last_results"] = res

    # unshard + epilogue: divide by denominator row, ELU (softmax row-scale
    # invariance makes the on-device v_i scaling drop out here)
    out = np.empty((B, N, H * D), dtype=np.float32)
    for c in range(NCORES):
        b, pair = divmod(c, 2)
        oc = res.results[c]["o"]  # [2, 65, N] f32
        for hl in range(2):
            num = oc[hl, 0:64, :]  # [64, N]
            den = oc[hl, 64, :]  # [N]
            z = (num / den).T  # [N, 64]
            col = (2 * pair + hl) * D
            out[b, :, col : col + D] = np.where(z > 0, z, np.expm1(z))
    return out
